# revision 20
# baseline (speedup 1.0000x reference)
"""BiGN (2-relation LightGCN-style GNN) on 8 Trainium2 NeuronCores — v2.

Strategy (dst-sharded SPMD, optimized for the SWDGE gather bottleneck):
- Node space: core k owns 18750 nodes, assigned to 150 blocks x 128 slots,
  6 stages of 25 blocks. Table layout = 6 chunks of [8 cores x 3200 rows];
  chunk == stage, so stage-sliced AllGathers unlock next-layer gathers
  incrementally. Layer-0 table fed as a replicated input (no first AG).
- Block assignment is degree-balanced per (relation, chunk) so per-
  (block, rel, chunk) edge groups pad to <=384 slots (less DGE/DVE work).
- Last layer computes only the ~8K sampled user/item rows (16x less work).
- One-hot dst matrices precomputed on host in fp8 and DMA'd (no DVE is_eq);
  PE matmuls use fp8 stationary x bf16 moving messages.
- PSUM drained via the scalar (ACT) engine; attention math on SBUF.
- Gathers round-robin 4 SWDGE queues (4 Q7 core pairs) with deep pools and
  supertile-ahead stream prefetch so descriptor generation runs 4-wide.
"""

import os
import numpy as np
import ml_dtypes

# ---------------------------------------------------------------- constants
NCORES = 8
N_USER = 100000
N_ITEM = 50000
N = N_USER + N_ITEM
D = 64
NPC_REAL = N // NCORES            # 18750
NB = 150                          # blocks per core
NSTAGES = 6
SBLK = NB // NSTAGES              # 25 blocks per stage
STAGE_ROWS = SBLK * 128           # 3200
NODES_PER_STAGE = NPC_REAL // NSTAGES  # 3125
CHUNK_ROWS = NCORES * STAGE_ROWS  # 25600
STILE = 8
NSTILES = (NB + STILE - 1) // STILE    # 19 (last has 6 blocks)
STILE3 = 6
TARGET = 384
BF16 = ml_dtypes.bfloat16
FP8 = ml_dtypes.float8_e4m3fn

_CACHE = {}


# ---------------------------------------------------------------- host prep
def _mk_targets():
    """Per (block-in-stage, si*6+ch) slot targets: mostly 384, with 7
    blocks per dim at 256 (spread across blocks) to cut padding."""
    T = np.full((SBLK, 12), 384, np.int64)
    for d in range(12):
        for k in range(7):
            T[(7 * d + k) % SBLK, d] = 256
    return T


def _assign_blocks(dv, nblocks, target, cap=128):
    """Greedy balance of nodes (rows of dv, 12-dim degree vectors) into
    nblocks blocks of <=cap nodes, keeping per-dim sums <= target[b, d]."""
    n = dv.shape[0]
    order = np.argsort(-dv.sum(1), kind="stable")
    counts = np.zeros((nblocks, dv.shape[1]), np.int64)
    fill = np.zeros(nblocks, np.int64)
    assign = np.zeros(n, np.int64)
    for i in order:
        v = dv[i]
        room = fill < cap
        ok = room & ((counts + v) <= target).all(1)
        cand = np.where(ok)[0]
        if len(cand) == 0:
            cand = np.where(room)[0]
        j = cand[np.argmin(((counts[cand] + v) - target[cand]).max(1))]
        assign[i] = j
        counts[j] += v
        fill[j] += 1
    return assign


def _wrap16(gi):
    gw = np.ascontiguousarray(gi.reshape(-1, 16).T)
    return np.tile(gw, (8, 1))


def _preprocess(inputs):
    gs = np.asarray(inputs["graph_src"]).astype(np.int64)
    gd = np.asarray(inputs["graph_dst"]).astype(np.int64)
    gv = np.asarray(inputs["graph_val"]).astype(np.float32)
    ss = np.asarray(inputs["sim_src"]).astype(np.int64)
    sd = np.asarray(inputs["sim_dst"]).astype(np.int64)
    sv = np.asarray(inputs["sim_val"]).astype(np.float32)
    users = np.asarray(inputs["users"]).astype(np.int64)
    items = np.asarray(inputs["items"]).astype(np.int64)

    core = np.arange(N) // NPC_REAL
    r = np.arange(N) % NPC_REAL
    stage = r // NODES_PER_STAGE          # 0..5

    # degree vectors as dst: dim = si*6 + src_stage
    dv = np.zeros((N, 12), np.int32)
    np.add.at(dv, (gd, 0 + stage[gs]), 1)
    np.add.at(dv, (sd, 6 + stage[ss]), 1)

    blk = np.zeros(N, np.int64)
    pos = np.zeros(N, np.int64)
    T = _mk_targets()
    for c in range(NCORES):
        base = c * NPC_REAL
        for s in range(NSTAGES):
            sel = np.arange(base + s * NODES_PER_STAGE,
                            base + (s + 1) * NODES_PER_STAGE)
            a = _assign_blocks(dv[sel], SBLK, T)
            blk[sel] = s * SBLK + a
            for b in range(SBLK):
                m = sel[a == b]
                pos[m] = np.arange(len(m))
    lidx = core * STAGE_ROWS + (blk % SBLK) * 128 + pos   # chunk-local row
    assert lidx.max() < CHUNK_ROWS

    # shared n_mm for big layers
    cnt = np.zeros((NCORES, 2, NB, NSTAGES), np.int64)
    np.add.at(cnt, (core[gd], 0, blk[gd], stage[gs]), 1)
    np.add.at(cnt, (core[sd], 1, blk[sd], stage[ss]), 1)
    n_mm = np.maximum(1, -(-cnt.max(axis=0) // 128))      # [2, NB, NSTAGES]

    # ---- sampled (last layer) node space
    uu = np.unique(users)
    ii = np.unique(items) + N_USER
    samp = np.concatenate([uu, ii])
    issamp = np.zeros(N, bool)
    issamp[samp] = True
    # per (core, chunk) sampled counts -> shared block allocation
    n3 = np.zeros((NCORES, NSTAGES), np.int64)
    np.add.at(n3, (core[samp], stage[samp]), 1)
    B3 = np.maximum(1, -(-n3.max(axis=0) // 128))         # blocks per chunk
    boff3 = np.concatenate([[0], np.cumsum(B3)])          # block offsets
    NB3 = int(boff3[-1])
    nst3 = (NB3 + STILE3 - 1) // STILE3
    # compact slot per sampled node: ordered by (core, chunk, lidx)
    slot3 = np.full(N, -1, np.int64)
    samp_by_core = []
    for c in range(NCORES):
        rows_c = []
        for s in range(NSTAGES):
            m = samp[(core[samp] == c) & (stage[samp] == s)]
            m = m[np.argsort(lidx[m], kind="stable")]
            slot3[m] = boff3[s] * 128 + np.arange(len(m))
            rows_c.append(m)
        samp_by_core.append(rows_c)
    blk3 = np.where(issamp, slot3 // 128, -1)

    # L3 edges (dst sampled)
    gm = issamp[gd]
    sm = issamp[sd]
    l3 = [(gs[gm], gd[gm], gv[gm]), (ss[sm], sd[sm], sv[sm])]
    cnt3 = np.zeros((NCORES, 2, NB3, NSTAGES), np.int64)
    for si, (es, ed, ev) in enumerate(l3):
        np.add.at(cnt3, (core[ed], si, blk3[ed], stage[es]), 1)
    n_mm3 = np.maximum(1, -(-cnt3.max(axis=0) // 128))    # [2, NB3, NSTAGES]

    # ---- build per-core streams
    def build_streams(edge_sets, nmm, nblocks, stile, nstiles, blkmap, posmap):
        """edge_sets: per si (src, dst, val); returns per-core dict + meta.

        Call order: st -> ch -> si. meta entries: (st, ch, si, M)."""
        meta = []
        for st in range(nstiles):
            blocks = range(st * stile, min((st + 1) * stile, nblocks))
            for ch in range(NSTAGES):
                for si in range(2):
                    M = int(sum(nmm[si, b, ch] for b in blocks))
                    meta.append((st, ch, si, M))
        streams = []
        for c in range(NCORES):
            sel = {}
            for si, (es, ed, ev) in enumerate(edge_sets):
                m = core[ed] == c
                key = blkmap[ed[m]] * NSTAGES + stage[es[m]]
                order = np.argsort(key, kind="stable")
                idx_sorted = np.nonzero(m)[0][order]
                key_sorted = key[order]
                starts = np.searchsorted(key_sorted,
                                         np.arange(nblocks * NSTAGES))
                ends = np.searchsorted(key_sorted,
                                       np.arange(nblocks * NSTAGES) + 1)
                sel[si] = (idx_sorted, starts, ends, es, ed, ev)
            gidx_p, val_p, oh_p = [], [], []
            for st in range(nstiles):
                blocks = range(st * stile, min((st + 1) * stile, nblocks))
                for ch in range(NSTAGES):
                    for si in range(2):
                        idx_sorted, starts, ends, es, ed, ev = sel[si]
                        gi_l, va_l, dp_l = [], [], []
                        for b in blocks:
                            k = b * NSTAGES + ch
                            eidx = idx_sorted[starts[k]:ends[k]]
                            L = int(nmm[si, b, ch]) * 128
                            assert len(eidx) <= L, (len(eidx), L)
                            gi = np.zeros(L, np.int16)
                            va = np.zeros(L, np.float32)
                            dp = np.full(L, -1, np.int64)
                            gi[:len(eidx)] = lidx[es[eidx]]
                            va[:len(eidx)] = ev[eidx]
                            dp[:len(eidx)] = posmap[ed[eidx]]
                            gi_l.append(gi)
                            va_l.append(va)
                            dp_l.append(dp)
                        gi = np.concatenate(gi_l)
                        va = np.concatenate(va_l)
                        dp = np.concatenate(dp_l)
                        L = len(gi)
                        M = L // 128
                        gidx_p.append(_wrap16(gi))
                        val_p.append(np.ascontiguousarray(
                            va.reshape(M, 128).T))
                        oh = np.zeros((128, M * 128), np.float32)
                        j = np.nonzero(dp >= 0)[0]
                        oh[j % 128, (j // 128) * 128 + dp[j]] = 1.0
                        oh_p.append(oh)
            streams.append(dict(
                gidx=np.concatenate(gidx_p, axis=1),
                val=np.concatenate(val_p, axis=1),
                oh=np.concatenate(oh_p, axis=1).astype(FP8),
            ))
        return meta, streams

    pos3 = np.where(issamp, slot3 % 128, 0)
    big_meta, big_streams = build_streams(
        [(gs, gd, gv), (ss, sd, sv)], n_mm, NB, STILE, NSTILES, blk, pos)
    l3_meta, l3_streams = build_streams(
        l3, n_mm3, NB3, STILE3, nst3, blk3, pos3)

    # sampled-row gather streams (same idx reused for t0/t1/t2)
    sgidx = []
    for c in range(NCORES):
        parts = []
        for s in range(NSTAGES):
            m = samp_by_core[c][s]
            L = int(B3[s]) * 128
            gi = np.zeros(L, np.int16)
            gi[:len(m)] = lidx[m]
            parts.append(_wrap16(gi))
        sgidx.append(np.concatenate(parts, axis=1))

    hostmeta = dict(core=core, blk=blk, pos=pos, lidx=lidx, stage=stage,
                    slot3=slot3, B3=B3, boff3=boff3, NB3=NB3, nst3=nst3,
                    n_mm=n_mm, n_mm3=n_mm3, samp=samp)
    return big_meta, big_streams, l3_meta, l3_streams, sgidx, hostmeta


def _table_inputs(emb0, hostmeta):
    """Per-core replicated t0 chunks + emb_own, in the device layout."""
    core = hostmeta["core"]
    blk = hostmeta["blk"]
    pos = hostmeta["pos"]
    t0 = [np.zeros((CHUNK_ROWS, D), np.float32) for _ in range(NSTAGES)]
    emb_own = np.zeros((NCORES, NB * 128, D), np.float32)
    # (emb_own cast to bf16 at the end — the device tile is bf16)
    s = blk // SBLK
    row = core * STAGE_ROWS + (blk % SBLK) * 128 + pos
    for st in range(NSTAGES):
        m = s == st
        t0[st][row[m]] = emb0[m]
    emb_own[core, blk * 128 + pos] = emb0
    return t0, emb_own.astype(BF16)


# ---------------------------------------------------------------- device
def _build_module(big_meta, l3_meta, hostmeta, bt16, bt128, lt16, lt128):
    import concourse.bacc as bacc
    import concourse.mybir as mybir
    import concourse.tile as tile
    from concourse.library_config import mlp

    f32 = mybir.dt.float32
    bf16 = mybir.dt.bfloat16
    fp8 = mybir.dt.float8e4
    i16 = mybir.dt.int16

    n_mm = hostmeta["n_mm"]
    n_mm3 = hostmeta["n_mm3"]
    B3 = hostmeta["B3"]
    boff3 = hostmeta["boff3"]
    NB3 = hostmeta["NB3"]
    nst3 = hostmeta["nst3"]
    st16 = NB3 * 8  # sgidx cols

    nc = bacc.Bacc("TRN2", target_bir_lowering=False, debug=False,
                   num_devices=NCORES, num_swdge_queues=4,
                   dynamic_dma_scratch_size=32768)

    emb_own_in = nc.dram_tensor("emb_own", [NB * 128, D], bf16,
                                kind="ExternalInput")
    t0_in = [nc.dram_tensor(f"t0_{s}", [CHUNK_ROWS, D], f32,
                            kind="ExternalInput") for s in range(NSTAGES)]
    bgidx = nc.dram_tensor("bgidx", [128, bt16], i16, kind="ExternalInput")
    bval = nc.dram_tensor("bval", [128, bt128], f32, kind="ExternalInput")
    boh = nc.dram_tensor("boh", [128, bt128 * 128], fp8,
                         kind="ExternalInput")
    lgidx = nc.dram_tensor("lgidx", [128, lt16], i16, kind="ExternalInput")
    lval = nc.dram_tensor("lval", [128, lt128], f32, kind="ExternalInput")
    loh = nc.dram_tensor("loh", [128, lt128 * 128], fp8,
                         kind="ExternalInput")
    sgidx = nc.dram_tensor("sgidx", [128, st16], i16, kind="ExternalInput")
    light_out = nc.dram_tensor("light_out", [NB3 * 128, D], f32,
                               kind="ExternalOutput")

    # per-supertile offsets into the big stream arrays (call order st,ch,si)
    def mk_offsets(meta, nstiles):
        per_st = [[] for _ in range(nstiles)]
        o16 = o128 = 0
        st_base = {}
        for (st, ch, si, M) in meta:
            if st not in st_base:
                st_base[st] = (o16, o128)
            per_st[st].append((ch, si, M, o16, o128))
            o16 += M * 8
            o128 += M
        return per_st, st_base, o16, o128

    big_per_st, big_base, tot16, tot128 = mk_offsets(big_meta, NSTILES)
    assert tot16 == bt16 and tot128 == bt128, (tot16, bt16, tot128, bt128)
    l3_per_st, l3_base, l3tot16, l3tot128 = mk_offsets(l3_meta, nst3)
    assert l3tot16 == lt16 and l3tot128 == lt128

    AG_FIRE = {(25 * (s + 1) - 1) // STILE: s for s in range(NSTAGES)}

    with tile.TileContext(nc) as tc:
        nc.gpsimd.load_library(mlp)
        with (
            tc.tile_pool(name="pers", bufs=1) as pers,
            tc.tile_pool(name="stream", bufs=3) as spool,
            tc.tile_pool(name="gath", bufs=5) as gpool,
            tc.tile_pool(name="msgs", bufs=4) as mpool,
            tc.tile_pool(name="oh", bufs=5) as opool,
            tc.tile_pool(name="att", bufs=2) as apool,
            tc.tile_pool(name="psum", bufs=2, space="PSUM") as ppool,
            tc.tile_pool(name="dram", bufs=1, space="DRAM") as dram,
            tc.tile_pool(name="dstage", bufs=1, space="DRAM") as dstage,
        ):
            emb_own = pers.tile([128, NB, D], bf16)
            nc.sync.dma_start(
                emb_own[:],
                emb_own_in[:].rearrange("(b p) d -> p b d", p=128))
            sgt = pers.tile([128, st16], i16)
            nc.sync.dma_start(sgt[:], sgidx[:])
            gsam = [pers.tile([128, NB3, D], f32, name=f"gsam{t}")
                    for t in range(3)]

            # internal tables (layers 1, 2) + stage inputs
            tch = {}
            stg = {}
            for l in (1, 2):
                tch[l] = [dram.tile([CHUNK_ROWS, D], f32, addr_space="Shared",
                                    name=f"t{l}_{s}") for s in range(NSTAGES)]
                stg[l] = [dstage.tile([STAGE_ROWS, D], f32,
                                      name=f"stg{l}_{s}")
                          for s in range(NSTAGES)]

            rrq = [0]

            # Hoisted num_idxs registers: one per distinct gather length so
            # each dma_gather call is 1 Pool instruction, not 2 (MOVE+gather).
            # The GpSimd engine queue is 8 deep; fewer instructions per call
            # means deeper gather lookahead and better Q7-pair overlap.
            lens = set()
            for (_, _, _, M) in big_meta + l3_meta:
                lens.add(M * 128)
            for s in range(NSTAGES):
                lens.add(int(B3[s]) * 128)
            lreg = {L: nc.gpsimd.to_reg(L) for L in sorted(lens)}

            def sample_gather_chunk(tables, t, s):
                Ls = int(B3[s]) * 128
                off = int(boff3[s])
                nc.gpsimd.dma_gather(
                    gsam[t][:, off:off + int(B3[s]), :],
                    tables[s][0:CHUNK_ROWS, :],
                    sgt[:, off * 8:off * 8 + Ls // 16],
                    Ls, lreg[Ls], D, single_packet=False,
                    queue_num=rrq[0])
                rrq[0] = (rrq[0] + 1) % 4

            def sample_gather(tables, t):
                for s in range(NSTAGES):
                    sample_gather_chunk(tables, t, s)

            def combine(ps0sl, ps1sl, nblk, eo):
                """Dual-relation attention combine; returns `new` (f32)."""
                s0 = apool.tile([128, nblk, D], f32, tag="s0")
                nc.scalar.activation(s0[:], ps0sl,
                                     mybir.ActivationFunctionType.Copy,
                                     scale=1.0)
                s1 = apool.tile([128, nblk, D], f32, tag="s1")
                nc.scalar.activation(s1[:], ps1sl,
                                     mybir.ActivationFunctionType.Copy,
                                     scale=1.0)
                e1 = apool.tile([128, nblk, D], f32, tag="e1")
                nc.vector.tensor_scalar_add(e1[:], eo, 1.0)
                att = []
                for t, sv_ in enumerate((s0, s1)):
                    tp = apool.tile([128, nblk, D], f32, tag=f"tp{t}")
                    nc.vector.tensor_tensor(tp[:], sv_[:], e1[:],
                                            mybir.AluOpType.mult)
                    red = apool.tile([128, nblk], f32, tag=f"red{t}")
                    nc.vector.tensor_reduce(red[:], tp[:],
                                            mybir.AxisListType.X,
                                            mybir.AluOpType.add)
                    a = apool.tile([128, nblk], f32, tag=f"att{t}")
                    nc.scalar.activation(a[:], red[:],
                                         mybir.ActivationFunctionType.Exp,
                                         scale=1.0 / D)
                    att.append(a)
                den = apool.tile([128, nblk], f32, tag="den")
                nc.vector.tensor_add(den[:], att[0][:], att[1][:])
                rec = apool.tile([128, nblk], f32, tag="rec")
                nc.vector.reciprocal(rec[:], den[:])
                w0 = apool.tile([128, nblk], f32, tag="w0")
                nc.vector.tensor_mul(w0[:], att[0][:], rec[:])
                w1 = apool.tile([128, nblk], f32, tag="w1")
                nc.vector.tensor_mul(w1[:], att[1][:], rec[:])
                t0m = apool.tile([128, nblk, D], f32, tag="t0m")
                nc.vector.tensor_tensor(
                    t0m[:], s0[:],
                    w0[:].unsqueeze(2).to_broadcast([128, nblk, D]),
                    mybir.AluOpType.mult)
                new = apool.tile([128, nblk, D], f32, tag="new")
                nc.vector.tensor_tensor(
                    new[:], s1[:],
                    w1[:].unsqueeze(2).to_broadcast([128, nblk, D]),
                    mybir.AluOpType.mult)
                nc.vector.tensor_add(new[:], new[:], t0m[:])
                return new

            def do_calls(per_st_entry, src_tables, stream_src, base16,
                         base128, ps, blocks):
                """Issue gathers + ms + matmuls for one supertile."""
                gt_l16, gt_l128 = 0, 0
                ent = per_st_entry
                st_L16 = sum(M * 8 for (_, _, M, _, _) in ent)
                st_L128 = sum(M for (_, _, M, _, _) in ent)
                gtile = spool.tile([128, st_L16], i16, tag="gidx")
                nc.sync.dma_start(
                    gtile[:], stream_src[0][:, base16:base16 + st_L16])
                vtile = spool.tile([128, st_L128], f32, tag="val")
                nc.sync.dma_start(
                    vtile[:], stream_src[1][:, base128:base128 + st_L128])
                nmm = stream_src[3]
                for (ch, si, M, o16, o128) in ent:
                    lo16 = o16 - base16
                    lo128 = o128 - base128
                    it = gtile[:, lo16:lo16 + M * 8]
                    gt = gpool.tile([128, M, D], f32, tag="gt")
                    nc.gpsimd.dma_gather(
                        gt[:], src_tables[ch][0:CHUNK_ROWS, :], it,
                        M * 128, lreg[M * 128], D, single_packet=False,
                        queue_num=rrq[0])
                    rrq[0] = (rrq[0] + 1) % 4
                    oht = opool.tile([128, M, 128], fp8, tag="oh")
                    nc.scalar.dma_start(
                        oht[:], stream_src[2][:, o128 * 128:(o128 + M) * 128])
                    mst = mpool.tile([128, M, D], bf16, tag="ms")
                    nc.vector.tensor_tensor(
                        mst[:], gt[:],
                        vtile[:, lo128:lo128 + M].unsqueeze(2)
                        .to_broadcast([128, M, D]),
                        mybir.AluOpType.mult)
                    m = 0
                    for bl, b in enumerate(blocks):
                        for _ in range(int(nmm[si, b, ch])):
                            nc.tensor.matmul(
                                ps[si][:, bl, :],
                                oht[:, m, :], mst[:, m, :],
                                start=False, stop=False,
                                skip_group_check=True)
                            m += 1
                    assert m == M, (m, M)

            # ---------------- L3 persistent state (accumulated during L1)
            lgt = pers.tile([128, lt16], i16)
            nc.sync.dma_start(lgt[:], lgidx[:])
            lvt = pers.tile([128, lt128], f32)
            nc.sync.dma_start(lvt[:], lval[:])
            ps3 = {}
            for si in range(2):
                ps3[si] = ppool.tile([128, NB3, D], f32,
                                     tag=f"ps3{si}", bufs=1, name=f"ps3{si}")
                nc.vector.memset(ps3[si][:], 0.0)
            l3_call_of = {}
            for st in range(nst3):
                for (ch, si, M, o16, o128) in l3_per_st[st]:
                    l3_call_of[(st, ch, si)] = (M, o16, o128)

            def issue_l3_chunk(ch):
                sample_gather_chunk(tch[2], 2, ch)
                for st in range(nst3):
                    blocks = list(range(st * STILE3,
                                        min((st + 1) * STILE3, NB3)))
                    for si in range(2):
                        M, o16, o128 = l3_call_of[(st, ch, si)]
                        gt = gpool.tile([128, M, D], f32, tag="gt")
                        nc.gpsimd.dma_gather(
                            gt[:], tch[2][ch][0:CHUNK_ROWS, :],
                            lgt[:, o16:o16 + M * 8],
                            M * 128, lreg[M * 128], D, single_packet=False,
                            queue_num=rrq[0])
                        rrq[0] = (rrq[0] + 1) % 4
                        oht = opool.tile([128, M, 128], fp8, tag="oh")
                        nc.scalar.dma_start(
                            oht[:], loh[:, o128 * 128:(o128 + M) * 128])
                        mst = mpool.tile([128, M, D], bf16, tag="ms")
                        nc.vector.tensor_tensor(
                            mst[:], gt[:],
                            lvt[:, o128:o128 + M].unsqueeze(2)
                            .to_broadcast([128, M, D]),
                            mybir.AluOpType.mult)
                        m = 0
                        for b in blocks:
                            for _ in range(int(n_mm3[si, b, ch])):
                                nc.tensor.matmul(
                                    ps3[si][:, b, :],
                                    oht[:, m, :], mst[:, m, :],
                                    start=False, stop=False,
                                    skip_group_check=True)
                                m += 1
                        assert m == M, (m, M)

            L3_AT = {}  # mid-layer L3 issue stalls on slow AGs; do all at end

            # ---------------- big layers 0, 1
            for layer in (0, 1):
                src_tables = t0_in if layer == 0 else tch[1]
                if layer == 0:
                    sample_gather(t0_in, 0)
                for st in range(NSTILES):
                    blocks = list(range(st * STILE,
                                        min((st + 1) * STILE, NB)))
                    nblk = len(blocks)
                    ps = {}
                    for si in range(2):
                        ps[si] = ppool.tile([128, STILE, D], f32,
                                            tag=f"ps{si}", name=f"ps{si}")
                        nc.vector.memset(ps[si][:], 0.0)
                    b16, b128 = big_base[st]
                    do_calls(big_per_st[st], src_tables,
                             (bgidx, bval, boh, n_mm), b16, b128, ps, blocks)
                    if layer == 1 and st == 0:
                        sample_gather(tch[1], 1)
                    if layer == 1 and st in L3_AT:
                        issue_l3_chunk(L3_AT[st])
                    eo = emb_own[:, blocks[0]:blocks[0] + nblk, :]
                    new = combine(ps[0][:, 0:nblk, :], ps[1][:, 0:nblk, :],
                                  nblk, eo)
                    if layer == 0:
                        nc.scalar.activation(
                            eo, new[:], mybir.ActivationFunctionType.Copy,
                            scale=1.0)
                    # layer 1: emb_own no longer needed afterwards
                    # stage writes (split at stage boundaries)
                    b0 = blocks[0]
                    b1 = blocks[-1] + 1
                    s0_ = b0 // SBLK
                    s1_ = (b1 - 1) // SBLK
                    for s in range(s0_, s1_ + 1):
                        lo = max(b0, s * SBLK)
                        hi = min(b1, (s + 1) * SBLK)
                        nc.sync.dma_start(
                            stg[layer + 1][s][:]
                            .rearrange("(b p) d -> p b d", p=128)
                            [:, lo - s * SBLK:hi - s * SBLK, :],
                            new[:, lo - b0:hi - b0, :])
                    if st in AG_FIRE:
                        s = AG_FIRE[st]
                        nc.gpsimd.collective_compute(
                            "AllGather", mybir.AluOpType.bypass,
                            ins=[stg[layer + 1][s].opt()],
                            outs=[tch[layer + 1][s].opt()],
                            replica_groups=[list(range(NCORES))])

            # ---------------- pruned last layer: all chunks + combines
            for ch in range(NSTAGES):
                issue_l3_chunk(ch)
            for st in range(nst3):
                blocks = list(range(st * STILE3, min((st + 1) * STILE3, NB3)))
                nblk = len(blocks)
                eo3 = gsam[2][:, blocks[0]:blocks[0] + nblk, :]
                new = combine(ps3[0][:, blocks[0]:blocks[0] + nblk, :],
                              ps3[1][:, blocks[0]:blocks[0] + nblk, :],
                              nblk, eo3)
                # light = (t0[s] + t1[s] + t2[s] + e3) / 4
                acc = apool.tile([128, nblk, D], f32, tag="acc")
                nc.vector.tensor_add(
                    acc[:], gsam[0][:, blocks[0]:blocks[0] + nblk, :],
                    gsam[1][:, blocks[0]:blocks[0] + nblk, :])
                nc.vector.tensor_add(acc[:], acc[:], eo3)
                nc.vector.tensor_add(acc[:], acc[:], new[:])
                fin = apool.tile([128, nblk, D], f32, tag="fin")
                nc.vector.tensor_scalar_mul(fin[:], acc[:], 0.25)
                nc.sync.dma_start(
                    light_out[:].rearrange("(b p) d -> p b d", p=128)
                    [:, blocks[0]:blocks[0] + nblk, :],
                    fin[:])
    nc.compile()
    return nc


# ---------------------------------------------------------------- entry
def _get_compiled(inputs):
    if "module" in _CACHE:
        return _CACHE["module"]
    big_meta, big_streams, l3_meta, l3_streams, sgidx, hostmeta = \
        _preprocess(inputs)
    bt16 = sum(M * 8 for (_, _, _, M) in big_meta)
    bt128 = sum(M for (_, _, _, M) in big_meta)
    lt16 = sum(M * 8 for (_, _, _, M) in l3_meta)
    lt128 = sum(M for (_, _, _, M) in l3_meta)
    nc = _build_module(big_meta, l3_meta, hostmeta, bt16, bt128, lt16, lt128)
    _CACHE["module"] = (nc, big_streams, l3_streams, sgidx, hostmeta)
    return _CACHE["module"]


def kernel(user_emb, item_emb, graph_src, graph_dst, graph_val,
           sim_src, sim_dst, sim_val, users, items):
    from concourse.bass_utils import run_bass_kernel_spmd
    import concourse.bass_utils as _bu
    trace = bool(int(os.environ.get("BIGN_TRACE", "0")))
    if trace:
        _bu.upload_artifacts = lambda tmpdir: tmpdir

    inputs = dict(user_emb=user_emb, item_emb=item_emb,
                  graph_src=graph_src, graph_dst=graph_dst,
                  graph_val=graph_val, sim_src=sim_src, sim_dst=sim_dst,
                  sim_val=sim_val, users=users, items=items)
    nc, big_streams, l3_streams, sgidx, hostmeta = _get_compiled(inputs)

    emb0 = np.concatenate([np.asarray(user_emb, np.float32),
                           np.asarray(item_emb, np.float32)], axis=0)
    t0, emb_own = _table_inputs(emb0, hostmeta)
    in_maps = []
    for c in range(NCORES):
        im = dict(emb_own=emb_own[c],
                  bgidx=big_streams[c]["gidx"], bval=big_streams[c]["val"],
                  boh=big_streams[c]["oh"],
                  lgidx=l3_streams[c]["gidx"], lval=l3_streams[c]["val"],
                  loh=l3_streams[c]["oh"], sgidx=sgidx[c])
        for s in range(NSTAGES):
            im[f"t0_{s}"] = t0[s]
        in_maps.append(im)

    res = run_bass_kernel_spmd(nc, in_maps, core_ids=list(range(NCORES)),
                               trace=trace)
    if trace and res.exec_time_ns is not None:
        kernel.last_exec_time_ns = res.exec_time_ns
        kernel.last_trace = res.instructions_and_trace

    # host assembly: sampled rows -> [B] dots
    slot3 = hostmeta["slot3"]
    core = hostmeta["core"]
    light = np.zeros((N, D), np.float32)
    samp = hostmeta["samp"]
    for c in range(NCORES):
        lo = res.results[c]["light_out"]
        m = samp[core[samp] == c]
        light[m] = lo[slot3[m]]
    u = np.asarray(users)
    it = np.asarray(items) + N_USER
    return (light[u] * light[it]).sum(axis=1).astype(np.float32)
